# revision 28
# baseline (speedup 1.0000x reference)
"""Trainium2 Bass kernel for the SD-style spatial attention block:

    y = x + out_w @ attn(qkv(groupnorm(x))) + out_b    (per sample)

x: [4, 256, 64, 64] fp32.  GroupNorm(8 groups) -> 1x1 conv QKV (4 heads,
head_dim 32, seq = 64*64 = 4096) -> softmax attention -> 1x1 out conv + bias
+ residual.

Sharding over 8 NeuronCores: core c handles batch b = c//2 and query-half
h = c%2 (2048 of the 4096 query positions).  Each core receives the full
sample as bf16 (for GroupNorm stats and K/V over all positions) plus its
bf16 query slice, and produces y[b][:, 2048*h : 2048*(h+1)] WITHOUT the x
residual; the host adds the exact fp32 residual while gathering.

v12: slots are (chunk, j-tile, head) with 1024-query chunks -- each exp
tile [128 keys, 1024 queries] is fed by ONE 1024-column S matmul and
consumed by ONE 1024-column PV matmul (half the matmul/LDWEIGHTS count of
the 512-query head-pair scheme, which left the PE with zero slack).
PSUM: S double buffer 4 banks + one 2-bank o_acc (2 chunks, serially
reused) + 2 scratch banks.  GroupNorm rstd via DVE Newton rsqrt keeps the
Exp table resident from t=0; K bias is dropped (cancels in softmax); the
denominator 1/D is broadcast row-aligned to o_acc by [128,32]-ones
matmuls so normalize is a plain elementwise multiply.
"""
import sys

sys.path.insert(0, "/opt/trn_rl_repo")

import numpy as np

import concourse.bass as bass
import concourse.bacc as bacc
import concourse.tile as tile
from concourse import mybir
from concourse.bass_utils import run_bass_kernel_spmd

F32 = mybir.dt.float32
I32 = mybir.dt.int32
BF16 = mybir.dt.bfloat16
FP16 = mybir.dt.float16
AF = mybir.ActivationFunctionType
OP = mybir.AluOpType

C = 256          # input channels
HID = 128        # qkv hidden (4 heads x 32)
NH = 4
HD = 32
SEQ = 4096       # 64*64 spatial positions
HALF = 2048      # query positions per core
G = 8            # groups
EPS = 1e-5
SCALE = float(HD) ** -0.5
ESHIFT = -2.0    # constant exp shift; cancels in O/D normalization

CW = 1024            # queries per chunk
N_IC = HALF // CW    # i-chunks per core (2)
N_JT = SEQ // 128    # j-tiles (32)
RSQRT_MAGIC = 0x5f3759df


def build_program():
    nc = bacc.Bacc()

    x_kv = nc.declare_dram_parameter("x_kv", [C, SEQ], BF16, isOutput=False)
    x_qb = nc.declare_dram_parameter("x_qb", [C, HALF], BF16, isOutput=False)
    wqkvT = nc.declare_dram_parameter("wqkvT", [C, 3 * HID], F32, isOutput=False)
    owbT = nc.declare_dram_parameter("owbT", [HID, C], BF16, isOutput=False)
    nb = nc.declare_dram_parameter("nb", [C, 1], F32, isOutput=False)
    ob = nc.declare_dram_parameter("ob", [C, 1], F32, isOutput=False)
    gsel = nc.declare_dram_parameter("gsel", [C, 128], F32, isOutput=False)
    gselTn = nc.declare_dram_parameter("gselTn", [128, C], F32, isOutput=False)
    y = nc.declare_dram_parameter("y", [C, HALF], F32, isOutput=True)

    with tile.TileContext(nc) as tc:
        import contextlib
        with contextlib.ExitStack() as ctx:
            persist = ctx.enter_context(tc.tile_pool(name="persist", bufs=1))

            # ---------------- persistent tiles ----------------
            wq_s = [persist.tile([128, 3 * HID], F32, tag=f"wqs{i}", name=f"wqs{i}") for i in range(2)]
            w2b = [persist.tile([128, 3 * HID], BF16, tag=f"w2b{i}", name=f"w2b{i}") for i in range(2)]
            ow_b = persist.tile([128, C], BF16, tag="owb", name="owb")
            gsel_t = [persist.tile([128, 128], F32, tag=f"gsel{i}", name=f"gsel{i}") for i in range(2)]
            gselTn_t = persist.tile([128, C], F32, tag="gselTn", name="gselTn")
            nb_t = [persist.tile([128, 1], F32, tag=f"nb{i}", name=f"nb{i}") for i in range(2)]
            ob_t = [persist.tile([128, 1], F32, tag=f"ob{i}", name=f"ob{i}") for i in range(2)]
            ones_h = persist.tile([128, 1], FP16, tag="ones", name="ones")
            ones32 = persist.tile([128, 32], FP16, tag="ones32", name="ones32")
            eps_t = persist.tile([128, 1], F32, tag="eps", name="eps")
            esh_t = persist.tile([128, 1], F32, tag="esh", name="esh")
            magic_t = persist.tile([128, 1], I32, tag="magic", name="magic")
            warm_t = persist.tile([128, 512], FP16, tag="warm", name="warm")
            a_t = [persist.tile([128, 1], F32, tag=f"a{i}", name=f"a{i}") for i in range(2)]
            b_t = [persist.tile([128, 1], F32, tag=f"b{i}", name=f"b{i}") for i in range(2)]
            nc.vector.memset(ones_h, 1.0)
            nc.vector.memset(ones32, 1.0)
            nc.vector.memset(eps_t, EPS)
            nc.vector.memset(esh_t, ESHIFT)
            nc.vector.memset(warm_t, 0.0)
            nc.gpsimd.memset(magic_t, RSQRT_MAGIC)

            # pin the Exp activation table NOW (only table this kernel uses)
            scrA = persist.tile([128, 1], F32, tag="scrA", name="scrA")
            nc.scalar.activation(out=scrA, in_=eps_t, func=AF.Exp)

            # ---------------- input DMA ----------------
            # x_kv bf16 [256, 4096] = 2MB gates the GroupNorm stats: it gets
            # strict priority on all 3 DMA queues; x_qb/weights ride behind.
            xkv = [persist.tile([128, SEQ], BF16, tag=f"xkv{i}", name=f"xkv{i}") for i in range(2)]
            xq = [persist.tile([128, HALF], BF16, tag=f"xq{i}", name=f"xq{i}") for i in range(2)]
            CH = 1024
            chunk_q = [
                ((0, 0), nc.sync), ((1, 0), nc.gpsimd), ((0, 1), nc.scalar),
                ((1, 1), nc.sync), ((0, 2), nc.gpsimd), ((1, 2), nc.scalar),
                ((0, 3), nc.sync), ((1, 3), nc.gpsimd),
            ]
            for (i, p), q in chunk_q:
                q.dma_start(out=xkv[i][:, CH * p:CH * (p + 1)],
                            in_=x_kv[128 * i:128 * (i + 1), CH * p:CH * (p + 1)])
            # query-half x: chunk-0 cols first (gate qproj0), rest later
            nc.sync.dma_start(out=xq[0][:, 0:CW], in_=x_qb[0:128, 0:CW])
            nc.scalar.dma_start(out=xq[1][:, 0:CW], in_=x_qb[128:256, 0:CW])
            for i in range(2):
                nc.gpsimd.dma_start(out=xq[i][:, CW:HALF],
                                    in_=x_qb[128 * i:128 * (i + 1), CW:HALF])
            # weights/consts
            nc.scalar.dma_start(out=wq_s[0], in_=wqkvT[0:128, :])
            nc.scalar.dma_start(out=wq_s[1], in_=wqkvT[128:256, :])
            nc.sync.dma_start(out=ow_b, in_=owbT[:, :])
            nc.gpsimd.dma_start(out=gselTn_t, in_=gselTn[:, :])
            for i in range(2):
                nc.sync.dma_start(out=gsel_t[i], in_=gsel[128 * i:128 * (i + 1), :])
                nc.gpsimd.dma_start(out=nb_t[i], in_=nb[128 * i:128 * (i + 1), :])
                nc.gpsimd.dma_start(out=ob_t[i], in_=ob[128 * i:128 * (i + 1), :])

            kq = persist.tile([128, SEQ], BF16, tag="K", name="K")
            qq = persist.tile([128, HALF], BF16, tag="Q", name="Q")
            vt_b = persist.tile([128, SEQ], FP16, tag="VT", name="VT")
            qkvb = [persist.tile([128, 1], F32, tag=f"qkvb{m}", name=f"qkvb{m}") for m in (0, 2)]
            qkvb = {0: qkvb[0], 2: qkvb[1]}
            # per-chunk softmax denominator accumulators [keys, (head, q)]
            dp = [persist.tile([128, NH * CW], FP16, tag=f"dp{i}", name=f"dp{i}") for i in range(2)]
            nc.gpsimd.memset(dp[0], 0.0)
            nc.gpsimd.memset(dp[1], 0.0)

            # ---------------- GroupNorm statistics ----------------
            with tc.tile_pool(name="gn", bufs=1) as gn, \
                 tc.tile_pool(name="ps", bufs=2, space="PSUM") as ps:
                # dummy matmuls keep the PE out of its low p-state while the
                # x DMA + stats gate the real work
                dps = ps.tile([128, 2048], F32, tag="ps", name="ps")

                def warm(n):
                    for w in range(n):
                        nc.tensor.matmul(dps[0:1, 512 * (w % 2):512 * (w % 2 + 1)],
                                         ones_h, warm_t, start=True, stop=True,
                                         skip_group_check=True)
                warm(36)

                # bn_stats in chunk-arrival order
                stats = [gn.tile([128, 8, 6], F32, tag=f"st{i}", name=f"st{i}") for i in range(2)]
                stat_order = [(0, 0), (0, 1), (1, 0), (1, 1),
                              (0, 2), (0, 3), (1, 2), (1, 3),
                              (0, 4), (0, 5), (1, 4), (1, 5),
                              (0, 6), (0, 7), (1, 6), (1, 7)]
                for i, s in stat_order:
                    nc.vector.bn_stats(out=stats[i][:, s, :],
                                       in_=xkv[i][:, 512 * s:512 * (s + 1)])
                pp = [gn.tile([128, 2], F32, tag=f"pp{i}", name=f"pp{i}") for i in range(2)]
                for i in range(2):
                    mv = gn.tile([128, 2], F32, tag=f"mv{i}", name=f"mv{i}")
                    nc.vector.bn_aggr(out=mv, in_=stats[i])
                    # pp = (mean, E[x^2]) per partition
                    tmp = gn.tile([128, 1], F32, tag=f"tmp{i}", name=f"tmp{i}")
                    nc.vector.tensor_copy(pp[i][:, 0:1], mv[:, 0:1])
                    nc.vector.tensor_mul(tmp, mv[:, 0:1], mv[:, 0:1])
                    nc.vector.tensor_add(pp[i][:, 1:2], mv[:, 1:2], tmp)

                # group sums: psum[g, :] = sum over channels of group g
                gs_ps = ps.tile([128, 2048], F32, tag="ps", name="ps")
                for i in range(2):
                    nc.tensor.matmul(gs_ps[:, 0:2], gsel_t[i], pp[i],
                                     start=(i == 0), stop=(i == 1))
                # keep the PE warm across the stats-math window
                warm(16)
                gsb = gn.tile([128, 2], F32, tag="gsb", name="gsb")
                nc.vector.tensor_scalar_mul(gsb, gs_ps[:, 0:2], 1.0 / 32.0)
                varg = gn.tile([128, 1], F32, tag="varg", name="varg")
                tmp2 = gn.tile([128, 1], F32, tag="tmp2", name="tmp2")
                nc.vector.tensor_mul(tmp2, gsb[:, 0:1], gsb[:, 0:1])
                nc.vector.tensor_sub(varg, gsb[:, 1:2], tmp2)
                nc.vector.tensor_scalar_add(varg, varg, EPS)
                # rstd = rsqrt(varg): bit-hack seed + 2 Newton iterations
                half_i = gn.tile([128, 1], I32, tag="halfi", name="halfi")
                y0b = gn.tile([128, 1], I32, tag="y0b", name="y0b")
                nc.vector.tensor_scalar(out=half_i, in0=varg.bitcast(I32),
                                        scalar1=1, scalar2=None,
                                        op0=OP.logical_shift_right)
                nc.vector.tensor_sub(y0b, magic_t, half_i)
                yk = y0b.bitcast(F32)
                rstd = gn.tile([128, 1], F32, tag="rstd", name="rstd")
                for it in range(2):
                    y2 = gn.tile([128, 1], F32, tag=f"y2_{it}", name=f"y2_{it}")
                    t_ = gn.tile([128, 1], F32, tag=f"t_{it}", name=f"t_{it}")
                    h_ = gn.tile([128, 1], F32, tag=f"h_{it}", name=f"h_{it}")
                    nxt = rstd if it == 1 else gn.tile([128, 1], F32, tag="y1", name="y1")
                    nc.vector.tensor_mul(y2, yk, yk)
                    nc.vector.tensor_mul(t_, varg, y2)
                    nc.vector.tensor_scalar(out=h_, in0=t_, scalar1=-0.5,
                                            scalar2=1.5, op0=OP.mult, op1=OP.add)
                    nc.vector.tensor_mul(nxt, yk, h_)
                    yk = nxt
                # gstats2 = (mean*rstd, rstd) per group-partition
                gstats = gn.tile([128, 2], F32, tag="gstats", name="gstats")
                nc.vector.tensor_mul(gstats[:, 0:1], gsb[:, 0:1], rstd)
                nc.vector.tensor_copy(gstats[:, 1:2], rstd)

                # broadcast to channels via nw-folded selector:
                # cs = (nw*mean*rstd, nw*rstd) ; a = cs1 ; b = nb - cs0
                for i in range(2):
                    cs_ps = ps.tile([128, 2048], F32, tag="ps", name="ps")
                    nc.tensor.matmul(cs_ps[:, 0:2], gselTn_t[:, 128 * i:128 * (i + 1)],
                                     gstats, start=True, stop=True)
                    nc.vector.tensor_copy(a_t[i], cs_ps[:, 1:2])
                    nc.vector.tensor_sub(b_t[i], nb_t[i], cs_ps[:, 0:1])

                # fold GroupNorm scale into QKV weights (bf16 out, one op)
                for i in range(2):
                    nc.vector.tensor_scalar(out=w2b[i], in0=wq_s[i],
                                            scalar1=a_t[i], scalar2=None,
                                            op0=OP.mult)
                # q bias (critical path); v bias emitted in the prologue;
                # k bias cancels in softmax
                bp = ps.tile([128, 2048], F32, tag="ps", name="ps")
                for i in range(2):
                    nc.tensor.matmul(bp[:, 0:1], wq_s[i][:, 0:128],
                                     b_t[i], start=(i == 0), stop=(i == 1))
                nc.vector.tensor_copy(qkvb[0], bp[:, 0:1])

            # ---------------- attention ----------------
            with (
                tc.tile_pool(name="sgp", bufs=2, space="PSUM") as sgp,
                tc.tile_pool(name="accp", bufs=1, space="PSUM") as accp,
                tc.tile_pool(name="finp", bufs=2, space="PSUM") as finp,
                tc.tile_pool(name="apool", bufs=4) as apool,
                tc.tile_pool(name="fin", bufs=2) as fin,
            ):
                slots = [(c, t, h) for c in range(N_IC) for t in range(N_JT)
                         for h in range(NH)]
                sg_of = {}
                acc_of = {}

                def emit_S(idx):
                    # matmul output must stay within one PSUM bank (512 f32
                    # cols), so the 1024-query tile is two matmuls sharing
                    # the same stationary kq strip
                    c, t, h = slots[idx]
                    sg = sgp.tile([128, CW], F32, tag="sg", name="sg")
                    for q2 in range(2):
                        nc.tensor.matmul(
                            sg[:, 512 * q2:512 * (q2 + 1)],
                            kq[32 * h:32 * (h + 1), 128 * t:128 * (t + 1)],
                            qq[32 * h:32 * (h + 1),
                               CW * c + 512 * q2:CW * c + 512 * (q2 + 1)],
                            start=True, stop=True, tile_position=(32 * h, 0),
                        )
                    sg_of[idx] = sg

                def emit_qproj(icb, use_act):
                    # one 512-col piece of the q projection (icb in halves)
                    qp = finp.tile([128, 512], F32, tag="fp", name="qp")
                    for i in range(2):
                        nc.tensor.matmul(qp, w2b[i][:, 0:HID],
                                         xq[i][:, 512 * icb:512 * (icb + 1)],
                                         start=(i == 0), stop=(i == 1))
                    dst = qq[:, 512 * icb:512 * (icb + 1)]
                    if use_act:
                        nc.scalar.activation(out=dst, in_=qp, func=AF.Identity,
                                             bias=qkvb[0], scale=1.0)
                    else:
                        nc.vector.tensor_scalar_add(dst, qp, qkvb[0])

                def emit_seg_K(seg):
                    # K' without bias (cancels in softmax); plain bf16 drain
                    sl = slice(512 * seg, 512 * (seg + 1))
                    kp = finp.tile([128, 512], F32, tag="fp", name="kp")
                    for i in range(2):
                        nc.tensor.matmul(kp, w2b[i][:, HID:2 * HID],
                                         xkv[i][:, sl], start=(i == 0), stop=(i == 1))
                    nc.vector.tensor_copy(kq[:, sl], kp)

                def emit_VT_group(g):
                    # V^T for j-tiles 4g..4g+3, one psum tile + one DVE drain
                    vtp = finp.tile([128, 512], F32, tag="fp", name="vtp")
                    for tt in range(4):
                        t = 4 * g + tt
                        for i in range(2):
                            nc.tensor.matmul(vtp[:, 128 * tt:128 * (tt + 1)],
                                             xkv[i][:, 128 * t:128 * (t + 1)],
                                             w2b[i][:, 2 * HID:3 * HID],
                                             start=(i == 0), stop=(i == 1))
                    nc.vector.tensor_copy(vt_b[:, 512 * g:512 * (g + 1)], vtp)

                fin_state = {}

                def emit_dmm(c, h, pc):
                    # D_h for 512 queries, broadcast over o_acc rows 32h..+32
                    key = (c, pc)
                    if key not in fin_state:
                        fin_state[key] = finp.tile([128, 512], F32, tag="fp",
                                                   name=f"d4_{c}_{pc}")
                    d4 = fin_state[key]
                    nc.tensor.matmul(
                        d4[32 * h:32 * (h + 1), :], ones32,
                        dp[c][:, CW * h + 512 * pc:CW * h + 512 * (pc + 1)],
                        start=True, stop=True,
                        tile_position=(0, 32 * h), skip_group_check=True,
                    )

                def emit_dchain(c, pc):
                    d4 = fin_state[(c, pc)]
                    dmx = fin.tile([128, 512], F32, tag="dmx", name="dmx")
                    nc.vector.tensor_scalar_max(dmx, d4, 1e-30)
                    dr32 = fin.tile([128, 512], F32, tag="dr32", name="dr32")
                    scr = fin.tile([128, 512], F32, tag="scr", name="scr")
                    nc.vector.reciprocal_approx_accurate(out=dr32, in_=dmx,
                                                         scratch=scr)
                    drb = fin.tile([128, 512], BF16, tag="drb", name="drb")
                    nc.vector.tensor_copy(drb, dr32)
                    fin_state[(c, pc)] = drb

                def emit_osb(c, pc):
                    o_sb = fin.tile([128, 512], F32, tag="osb", name="osb")
                    nc.vector.tensor_copy(o_sb, acc_of[c][:, 512 * pc:512 * (pc + 1)])
                    fin_state[("o", c, pc)] = o_sb

                def finalize_piece(c, pc):
                    drb = fin_state[(c, pc)]
                    o_sb = fin_state[("o", c, pc)]
                    on32 = fin.tile([128, 512], F32, tag="on32", name="on32")
                    on_b = fin.tile([128, 512], BF16, tag="onb", name="onb")
                    nc.vector.tensor_mul(on32, o_sb, drb)
                    nc.vector.tensor_scalar_add(on_b, on32, qkvb[2])
                    for oc in range(2):
                        fo = finp.tile([128, 512], F32, tag="fp", name="fo")
                        nc.tensor.matmul(fo, ow_b[:, 128 * oc:128 * (oc + 1)],
                                         on_b, start=True, stop=True)
                        ysb = fin.tile([128, 512], F32, tag="ysb", name="ysb")
                        nc.vector.tensor_scalar_add(ysb, fo, ob_t[oc])
                        q_eng = nc.sync if oc == 0 else nc.gpsimd
                        q_eng.dma_start(
                            out=y[128 * oc:128 * (oc + 1),
                                  CW * c + 512 * pc:CW * c + 512 * (pc + 1)],
                            in_=ysb,
                        )

                def emit_PV(idx, a_t2):
                    c, t, h = slots[idx]
                    for q2 in range(2):
                        nc.tensor.matmul(
                            acc_of[c][32 * h:32 * (h + 1),
                                      512 * q2:512 * (q2 + 1)],
                            vt_b[:, 128 * t + 32 * h:128 * t + 32 * (h + 1)],
                            a_t2[:, 512 * q2:512 * (q2 + 1)],
                            start=(t == 0), stop=(t == N_JT - 1),
                            tile_position=(0, 32 * h), skip_group_check=True,
                        )

                # prologue: K seg0 + q chunk 0 gate the first S/exp; VT group
                # 0 is needed first by PV(0), after exp(0)
                emit_seg_K(0)
                emit_qproj(0, use_act=True)
                emit_qproj(1, use_act=True)
                emit_S(0)
                emit_VT_group(0)
                # v bias (first used at finalize of chunk 0)
                vbp = finp.tile([128, 512], F32, tag="fp", name="vbp")
                for i in range(2):
                    nc.tensor.matmul(vbp[:, 0:1], wq_s[i][:, 256:384],
                                     b_t[i], start=(i == 0), stop=(i == 1))
                nc.vector.tensor_copy(qkvb[2], vbp[:, 0:1])

                a_of = {}
                for idx, (c, t, h) in enumerate(slots):
                    if t == 0 and h == 0 and c not in acc_of:
                        acc_of[c] = accp.tile([128, CW], F32, tag="Oacc", name="Oacc")

                    sg = sg_of.pop(idx)
                    a_t2 = apool.tile([128, CW], FP16, tag="A", name="A")
                    a_of[idx] = a_t2
                    nc.scalar.activation(out=a_t2, in_=sg, func=AF.Exp,
                                         scale=SCALE, bias=esh_t)
                    # S(idx+1) goes on the PE queue BEFORE the delayed PV so
                    # ScalarE never waits on the PE.
                    if idx + 1 < len(slots):
                        emit_S(idx + 1)
                    # chunk-boundary finalize staggering (chunk c-1 -> c):
                    # o_sb copies must be emitted BEFORE chunk c's first PVs
                    # (single o_acc buffer).
                    if c > 0 and t == 0:
                        if h == 0:
                            emit_dmm(c - 1, 3, 0)
                            emit_dmm(c - 1, 3, 1)
                        elif h == 1:
                            emit_osb(c - 1, 0)
                            emit_osb(c - 1, 1)
                            emit_dchain(c - 1, 0)
                        elif h == 2:
                            emit_dchain(c - 1, 1)
                            finalize_piece(c - 1, 0)
                        elif h == 3:
                            finalize_piece(c - 1, 1)
                    if idx > 0:
                        emit_PV(idx - 1, a_of.pop(idx - 1))
                    nc.vector.tensor_add(dp[c][:, CW * h:CW * (h + 1)],
                                         dp[c][:, CW * h:CW * (h + 1)],
                                         a_t2)
                    # denominator matmuls for heads 0..2 as their dp completes
                    if t == N_JT - 1 and h >= 1:
                        emit_dmm(c, h - 1, 0)
                        emit_dmm(c, h - 1, 1)
                    # projection spreading (all inside chunk 0)
                    if c == 0 and h == 0 and t == 8:
                        emit_qproj(2, use_act=False)
                    if c == 0 and h == 2 and t == 8:
                        emit_qproj(3, use_act=False)
                    if c == 0 and h == 2 and t % 4 == 2 and t // 4 + 1 < 8:
                        emit_seg_K(t // 4 + 1)
                    if c == 0 and h == 3 and t % 4 == 3 and t // 4 + 1 < 8:
                        emit_VT_group(t // 4 + 1)
                n_last = len(slots) - 1
                emit_PV(n_last, a_of.pop(n_last))
                cl = N_IC - 1
                emit_dmm(cl, 3, 0)
                emit_dmm(cl, 3, 1)
                emit_osb(cl, 0)
                emit_osb(cl, 1)
                emit_dchain(cl, 0)
                emit_dchain(cl, 1)
                finalize_piece(cl, 0)
                finalize_piece(cl, 1)
    nc.compile()
    return nc


_NC_CACHE = {}


def _get_nc():
    if "nc" not in _NC_CACHE:
        _NC_CACHE["nc"] = build_program()
    return _NC_CACHE["nc"]


def _host_inputs(x, norm_w, norm_b, qkv_w, out_w, out_b):
    """Build the 8 per-core input maps."""
    import ml_dtypes
    x = np.asarray(x, dtype=np.float32)
    B = x.shape[0]
    xf = x.reshape(B, C, SEQ)
    xb = [np.ascontiguousarray(xf[b].astype(ml_dtypes.bfloat16)) for b in range(B)]

    wqkvT = np.ascontiguousarray(np.asarray(qkv_w, np.float32).T)      # [256, 384]
    owbT = np.ascontiguousarray(
        np.asarray(out_w, np.float32).T.astype(ml_dtypes.bfloat16))    # [128, 256]
    nw = np.asarray(norm_w, np.float32).reshape(C)
    nbv = np.asarray(norm_b, np.float32).reshape(C, 1).copy()
    obv = np.asarray(out_b, np.float32).reshape(C, 1).copy()

    gsel = np.zeros((C, 128), np.float32)
    for ch in range(C):
        gsel[ch, ch // 32] = 1.0
    gselTn = np.zeros((128, C), np.float32)
    for ch in range(C):
        gselTn[ch // 32, ch] = nw[ch]

    in_maps = []
    for core in range(8):
        b, h = core // 2, core % 2
        in_maps.append({
            "x_kv": xb[b],
            "x_qb": np.ascontiguousarray(xb[b][:, HALF * h:HALF * (h + 1)]),
            "wqkvT": wqkvT, "owbT": owbT, "nb": nbv, "ob": obv,
            "gsel": gsel, "gselTn": gselTn,
        })
    return in_maps


def run(x, norm_w, norm_b, qkv_w, out_w, out_b, trace=False, tmpdir=None):
    """Run on 8 cores; returns (y_full, BassKernelResults)."""
    nc = _get_nc()
    in_maps = _host_inputs(x, norm_w, norm_b, qkv_w, out_w, out_b)
    res = run_bass_kernel_spmd(nc, in_maps, core_ids=list(range(8)), trace=trace,
                               tmpdir=tmpdir)
    x = np.asarray(x, dtype=np.float32)
    B = x.shape[0]
    HW_SIDE = int(np.sqrt(SEQ))
    out = np.empty((B, C, SEQ), np.float32)
    for core in range(8):
        b, h = core // 2, core % 2
        out[b][:, HALF * h:HALF * (h + 1)] = res.results[core]["y"]
    # exact fp32 residual added on host (kernel output excludes x)
    out += x.reshape(B, C, SEQ)
    return out.reshape(B, C, HW_SIDE, HW_SIDE), res


def kernel(x, norm_w, norm_b, qkv_w, out_w, out_b):
    y, _ = run(x, norm_w, norm_b, qkv_w, out_w, out_b, trace=False)
    return y
